# revision 1
# baseline (speedup 1.0000x reference)
"""Trainium2 Bass kernel for a relational GCN layer — dense count-matmul version.

Math (reference):
  S = feat[src]; msgs[e] = edge_nn(S[e], W_rel[rel[e]]) (tied 2-layer relu MLP)
  agg = segment_sum(msgs, dst, N); hn = LSTM-step(agg); out = MLP(hn)

Messages depend only on (rel, src): H[r, s] = edge_nn(feat[s], W_rel[r]) has
NUM_REL*N = 20k rows << E = 320k.  agg[n] = sum_{r,s} C[n, (r,s)] * H[(r,s)]
where C is the per-node edge-count matrix.  The baseline routed edges with a
GPSIMD SWDGE gather (~8 ns/edge, serial: ~390us/core).  This version instead
aggregates with dense count matmuls on the PE: C is built host-side, streamed
from HBM as fp8 (counts are small ints, exact in fp8), and multiplied against
the bf16 H table that phase A produces directly in SBUF (H never leaves the
chip).  No SWDGE gather at all.

Distribution: edges sharded by DESTINATION range (core c owns dst in
[1250c, 1250(c+1))); every core computes the full H table locally.
"""

import math
import numpy as np
import ml_dtypes

import concourse.bacc as bacc
import concourse.bass as bass
import concourse.mybir as mybir
import concourse.tile as tile
from concourse import bass_utils
from concourse.masks import make_identity

# ---- problem constants (hardcoded per spec) ----
N_NODES = 10000
N_EDGES = 320000
D = 256
D_OUT = 256
NUM_REL = 2
NCORES = 8
NPC = N_NODES // NCORES          # 1250 dst nodes per core
NBLK = math.ceil(NPC / 128)      # 10 dst blocks per core (last has 98 rows)
NPAD = 10240                     # src nodes padded to 20 chunks of 512
NROWS = NUM_REL * NPAD           # 20480 H-table rows
NKT = NROWS // 128               # 160 k-tiles
NDST = NBLK * 128                # 1280 padded dst cols per core

f32 = mybir.dt.float32
bf16 = mybir.dt.bfloat16
fp8 = mybir.dt.float8e4

_np_bf16 = ml_dtypes.bfloat16
_np_fp8 = ml_dtypes.float8_e4m3


# ----------------------------------------------------------------------------
# host-side preprocessing
# ----------------------------------------------------------------------------

def _prep_counts(src, dst, rel):
    """Per-core dense count matrix C [NROWS, NDST] in fp8 (exact small ints).

    Row id = rel*NPAD + src ; col id = dst % NPC (within the owning core).
    """
    row = rel.astype(np.int64) * NPAD + src.astype(np.int64)
    core = dst // NPC
    col = (dst % NPC).astype(np.int64)
    counts = np.zeros((NCORES, NROWS, NDST), dtype=np.uint8)
    np.add.at(counts, (core, row, col), 1)
    return [np.ascontiguousarray(counts[c]).astype(_np_fp8) for c in range(NCORES)]


def _prep_weights(inputs):
    feat = np.asarray(inputs["feat"], dtype=np.float32)
    W_rel = np.asarray(inputs["W_rel"], dtype=np.float32)
    b_rel = np.asarray(inputs["b_rel"], dtype=np.float32)
    W_ih = np.asarray(inputs["W_ih"], dtype=np.float32)
    b_ih = np.asarray(inputs["b_ih"], dtype=np.float32)
    b_hh = np.asarray(inputs["b_hh"], dtype=np.float32)
    W1 = np.asarray(inputs["W1"], dtype=np.float32)
    W2 = np.asarray(inputs["W2"], dtype=np.float32)
    W3 = np.asarray(inputs["W3"], dtype=np.float32)

    featT = np.zeros((D, NPAD), dtype=np.float32)
    featT[:, :N_NODES] = feat.T
    keep = np.r_[0:256, 512:1024]  # i, g, o gate columns (f unused: c0 = 0)
    com = {
        "featT": featT.astype(_np_bf16),
        "W_rT": np.ascontiguousarray(np.transpose(W_rel, (0, 2, 1))).astype(_np_bf16),
        "b_r_col": np.ascontiguousarray(b_rel[:, :, None]),                  # f32
        "b_r_rep": np.ascontiguousarray(
            np.broadcast_to(b_rel[:, None, :], (NUM_REL, 128, D))).copy(),   # f32
        "W_ihT": np.ascontiguousarray(W_ih.T[:, keep]).astype(np.float32),   # [256,768]
        "b_g_rep": np.ascontiguousarray(np.broadcast_to(
            (b_ih + b_hh)[keep][None, :], (128, 768))).astype(np.float32),
        "W1T": np.ascontiguousarray(W1.T).astype(_np_bf16),                  # [256,128]
        "b1_col": np.ascontiguousarray(inputs["b1"][:, None]).astype(np.float32),
        "W2T": np.ascontiguousarray(W2.T).astype(_np_bf16),                  # [128,128]
        "b2_col": np.ascontiguousarray(inputs["b2"][:, None]).astype(np.float32),
        "W3T": np.ascontiguousarray(W3.T).astype(_np_bf16),                  # [128,256]
        "b3_col": np.ascontiguousarray(
            np.asarray(inputs["b3"]).reshape(2, 128, 1)).astype(np.float32),
    }
    return com


# ----------------------------------------------------------------------------
# kernel builder
# ----------------------------------------------------------------------------

def _build():
    Relu = mybir.ActivationFunctionType.Relu
    Sig = mybir.ActivationFunctionType.Sigmoid
    Tanh = mybir.ActivationFunctionType.Tanh

    nc = bacc.Bacc("TRN2", target_bir_lowering=False, debug=False)

    featT_d = nc.dram_tensor("featT", [D, NPAD], bf16, kind="ExternalInput")
    W_rT_d = nc.dram_tensor("W_rT", [NUM_REL, D, D], bf16, kind="ExternalInput")
    b_r_col_d = nc.dram_tensor("b_r_col", [NUM_REL, D, 1], f32, kind="ExternalInput")
    b_r_rep_d = nc.dram_tensor("b_r_rep", [NUM_REL, 128, D], f32, kind="ExternalInput")
    W_ihT_d = nc.dram_tensor("W_ihT", [D, 768], mybir.dt.float32r, kind="ExternalInput")
    b_g_rep_d = nc.dram_tensor("b_g_rep", [128, 768], f32, kind="ExternalInput")
    W1T_d = nc.dram_tensor("W1T", [D, 128], bf16, kind="ExternalInput")
    b1_col_d = nc.dram_tensor("b1_col", [128, 1], f32, kind="ExternalInput")
    W2T_d = nc.dram_tensor("W2T", [128, 128], bf16, kind="ExternalInput")
    b2_col_d = nc.dram_tensor("b2_col", [128, 1], f32, kind="ExternalInput")
    W3T_d = nc.dram_tensor("W3T", [128, D_OUT], bf16, kind="ExternalInput")
    b3_col_d = nc.dram_tensor("b3_col", [2, 128, 1], f32, kind="ExternalInput")
    C_d = nc.dram_tensor("C", [NROWS, NDST], fp8, kind="ExternalInput")

    outT_d = nc.dram_tensor("outT", [D_OUT, NPC], f32, kind="ExternalOutput")

    with tile.TileContext(nc) as tc:
        with (
            tc.tile_pool(name="const", bufs=1) as cp,
            tc.tile_pool(name="work", bufs=3) as wp,
            tc.tile_pool(name="hbig", bufs=1) as hp_pool,
            tc.tile_pool(name="aggpool", bufs=1) as ap_pool,
        ):
            # ---- load constants to SBUF ----
            W_rT_sb = {}
            for r in range(NUM_REL):
                for h in range(2):
                    t = cp.tile([128, D], bf16, tag=f"wrt{r}{h}")
                    nc.sync.dma_start(t[:], W_rT_d[r, h * 128:(h + 1) * 128, :])
                    W_rT_sb[r, h] = t
            b_r_col_sb = {}
            for r in range(NUM_REL):
                for h in range(2):
                    t = cp.tile([128, 1], f32, tag=f"brc{r}{h}")
                    nc.sync.dma_start(t[:], b_r_col_d[r, h * 128:(h + 1) * 128, :])
                    b_r_col_sb[r, h] = t
            b_r_rep_sb = {}
            for r in range(NUM_REL):
                t = cp.tile([128, D], f32, tag=f"brr{r}")
                nc.sync.dma_start(t[:], b_r_rep_d[r, :, :])
                b_r_rep_sb[r] = t
            ident = cp.tile([128, 128], f32, tag="ident")
            make_identity(nc, ident[:])
            W_ihT_sb = {}
            for h in range(2):
                t = cp.tile([128, 768], mybir.dt.float32r, tag=f"wih{h}")
                nc.sync.dma_start(t[:], W_ihT_d[h * 128:(h + 1) * 128, :])
                W_ihT_sb[h] = t
            b_g_rep_sb = cp.tile([128, 768], f32, tag="bg")
            nc.sync.dma_start(b_g_rep_sb[:], b_g_rep_d[:, :])
            W1T_sb = {}
            for h in range(2):
                t = cp.tile([128, 128], bf16, tag=f"w1t{h}")
                nc.sync.dma_start(t[:], W1T_d[h * 128:(h + 1) * 128, :])
                W1T_sb[h] = t
            b1_col_sb = cp.tile([128, 1], f32, tag="b1")
            nc.sync.dma_start(b1_col_sb[:], b1_col_d[:, :])
            W2T_sb = cp.tile([128, 128], bf16, tag="w2t")
            nc.sync.dma_start(W2T_sb[:], W2T_d[:, :])
            b2_col_sb = cp.tile([128, 1], f32, tag="b2")
            nc.sync.dma_start(b2_col_sb[:], b2_col_d[:, :])
            W3T_sb = cp.tile([128, D_OUT], bf16, tag="w3t")
            nc.sync.dma_start(W3T_sb[:], W3T_d[:, :])
            b3_col_sb = {}
            for h in range(2):
                t = cp.tile([128, 1], f32, tag=f"b3{h}")
                nc.sync.dma_start(t[:], b3_col_d[h, :, :])
                b3_col_sb[h] = t

            # ---- H table, built in SBUF by phase A and consumed by the agg ----
            hbuf = hp_pool.tile([128, NKT, D], bf16, tag="hbuf")

            # agg psum: aggT[feat, dst] in 2 feat-halves x 3 dst chunks
            CHUNKS = [(0, 512), (512, 512), (1024, 256)]
            with tc.tile_pool(name="psAgg", bufs=1, space="PSUM") as psAgg:
                aggp = {}
                for h in range(2):
                    for ci, (c0, cw) in enumerate(CHUNKS):
                        aggp[h, ci] = psAgg.tile([128, cw], f32,
                                                 tag=f"agg{h}{ci}",
                                                 space="PSUM",
                                                 name=f"agg{h}{ci}")

                with tc.tile_pool(name="psA", bufs=1, space="PSUM") as psA:
                    # ---- phase A: H[r*NPAD + s] = edge_nn(feat[s], W_rel[r]) ----
                    for r in range(NUM_REL):
                        for chunk in range(NPAD // 512):
                            c0 = chunk * 512
                            ft = {}
                            for h in range(2):
                                t = wp.tile([128, 512], bf16, tag=f"ft{h}")
                                nc.sync.dma_start(
                                    t[:],
                                    featT_d[h * 128:(h + 1) * 128, c0:c0 + 512])
                                ft[h] = t
                            z1s = {}
                            for do_h in range(2):
                                z1p = psA.tile([128, 512], f32, tag="z1",
                                               space="PSUM", bufs=1)
                                for di_h in range(2):
                                    nc.tensor.matmul(
                                        z1p[:],
                                        lhsT=W_rT_sb[r, di_h][
                                            :, do_h * 128:(do_h + 1) * 128],
                                        rhs=ft[di_h][:],
                                        start=(di_h == 0), stop=(di_h == 1))
                                z = wp.tile([128, 512], bf16, tag=f"z1s{do_h}")
                                nc.scalar.activation(z[:], z1p[:], Relu,
                                                     bias=b_r_col_sb[r, do_h][:],
                                                     scale=1.0)
                                z1s[do_h] = z
                            for c4 in range(4):
                                kt = r * (NKT // 2) + chunk * 4 + c4
                                hp = psA.tile([128, D], f32, tag="hp",
                                              space="PSUM", bufs=1)
                                sl = slice(c4 * 128, (c4 + 1) * 128)
                                nc.tensor.matmul(hp[:], lhsT=z1s[0][:, sl],
                                                 rhs=W_rT_sb[r, 0][:],
                                                 start=True, stop=False)
                                nc.tensor.matmul(hp[:], lhsT=z1s[1][:, sl],
                                                 rhs=W_rT_sb[r, 1][:],
                                                 start=False, stop=True)
                                nc.vector.tensor_add(hp[:], hp[:],
                                                     b_r_rep_sb[r][:])
                                nc.scalar.activation(hbuf[:, kt, :], hp[:],
                                                     Relu, bias=0.0, scale=1.0)

                    # ---- agg: aggT[h] += H_k[:, h]^T @ C_k over all k ----
                    for k in range(NKT):
                        ct = wp.tile([128, NDST], fp8, tag="ct", bufs=6)
                        nc.scalar.dma_start(ct[:], C_d[k * 128:(k + 1) * 128, :])
                        for h in range(2):
                            for ci, (c0, cw) in enumerate(CHUNKS):
                                nc.tensor.matmul(
                                    aggp[h, ci][:],
                                    lhsT=hbuf[:, k, h * 128:(h + 1) * 128],
                                    rhs=ct[:, c0:c0 + cw],
                                    start=(k == 0), stop=(k == NKT - 1))

                # aggT in SBUF: [feat-half 128][NDST], f32 (used as f32r)
                aggT_sb = {}
                for h in range(2):
                    aggT_sb[h] = ap_pool.tile([128, NDST], mybir.dt.float32r, tag=f"aggT{h}", name=f"aggT{h}")
                    for ci, (c0, cw) in enumerate(CHUNKS):
                        nc.vector.tensor_copy(aggT_sb[h][:, c0:c0 + cw],
                                              aggp[h, ci][:])

            # ---- phase C: LSTM (single step from zero state) + MLP ----
            f32r = mybir.dt.float32r
            with tc.tile_pool(name="psC", bufs=1, space="PSUM") as psC:
                for b in range(NBLK):
                    nn = min(128, NPC - b * 128)
                    bsl = slice(b * 128, (b + 1) * 128)
                    cbG = psC.tile([128, 512], f32, tag="cbG", space="PSUM",
                                   bufs=2)
                    cbT = psC.tile([128, 512], f32, tag="cbT", space="PSUM",
                                   bufs=2)
                    cbM = psC.tile([128, 512], f32, tag="cbM", space="PSUM",
                                   bufs=2)
                    # i gates in cbG[0:256]
                    for h in range(2):
                        nc.tensor.matmul(
                            cbG[:, 0:256],
                            lhsT=aggT_sb[h][:, bsl],
                            rhs=W_ihT_sb[h][:, 0:256],
                            start=(h == 0), stop=(h == 1))
                    nc.vector.tensor_add(cbG[:, 0:256], cbG[:, 0:256],
                                         b_g_rep_sb[:, 0:256])
                    si = wp.tile([128, 256], f32, tag="si")
                    nc.scalar.activation(si[:], cbG[:, 0:256], Sig, bias=0.0,
                                         scale=1.0)
                    # g gates in cbG[256:512]
                    for h in range(2):
                        nc.tensor.matmul(
                            cbG[:, 256:512],
                            lhsT=aggT_sb[h][:, bsl],
                            rhs=W_ihT_sb[h][:, 256:512],
                            start=(h == 0), stop=(h == 1))
                    nc.vector.tensor_add(cbG[:, 256:512], cbG[:, 256:512],
                                         b_g_rep_sb[:, 256:512])
                    tg = wp.tile([128, 256], f32, tag="tg")
                    nc.scalar.activation(tg[:], cbG[:, 256:512], Tanh, bias=0.0,
                                         scale=1.0)
                    # o gates reuse cbG[256:512] (tile-granular ordering
                    # serializes the reuse after tg's read)
                    for h in range(2):
                        nc.tensor.matmul(
                            cbG[:, 256:512],
                            lhsT=aggT_sb[h][:, bsl],
                            rhs=W_ihT_sb[h][:, 512:768],
                            start=(h == 0), stop=(h == 1))
                    nc.vector.tensor_add(cbG[:, 256:512], cbG[:, 256:512],
                                         b_g_rep_sb[:, 512:768])
                    so = wp.tile([128, 256], f32, tag="so")
                    nc.scalar.activation(so[:], cbG[:, 256:512], Sig, bias=0.0,
                                         scale=1.0)
                    cc = wp.tile([128, 256], f32, tag="cc")
                    nc.vector.tensor_mul(cc[:], si[:], tg[:])
                    tcc = wp.tile([128, 256], f32, tag="tcc")
                    nc.scalar.activation(tcc[:], cc[:], Tanh, bias=0.0,
                                         scale=1.0)
                    hn = wp.tile([128, 256], f32, tag="hn")
                    nc.vector.tensor_mul(hn[:], so[:], tcc[:])
                    hnT = {}
                    for h in range(2):
                        dst_sl = slice(h * 128, (h + 1) * 128)
                        nc.tensor.transpose(cbT[:, dst_sl],
                                            hn[:, h * 128:(h + 1) * 128],
                                            ident[:])
                        ht = wp.tile([128, 128], bf16, tag=f"hnT{h}")
                        nc.vector.tensor_copy(ht[:], cbT[:, dst_sl])
                        hnT[h] = ht
                    # MLP (transposed activation layout: [feature, node])
                    for h in range(2):
                        nc.tensor.matmul(cbM[:, 0:128], lhsT=W1T_sb[h][:],
                                         rhs=hnT[h][:],
                                         start=(h == 0), stop=(h == 1))
                    x1s = wp.tile([128, 128], bf16, tag="x1s")
                    nc.scalar.activation(x1s[:], cbM[:, 0:128], Relu,
                                         bias=b1_col_sb[:], scale=1.0)
                    nc.tensor.matmul(cbM[:, 128:256], lhsT=W2T_sb[:],
                                     rhs=x1s[:], start=True, stop=True)
                    x2s = wp.tile([128, 128], bf16, tag="x2s")
                    nc.scalar.activation(x2s[:], cbM[:, 128:256], Relu,
                                         bias=b2_col_sb[:], scale=1.0)
                    for oh in range(2):
                        nc.tensor.matmul(cbM[:, 256 + oh * 128:384 + oh * 128],
                                         lhsT=W3T_sb[:, oh * 128:(oh + 1) * 128],
                                         rhs=x2s[:], start=True, stop=True)
                        osb = wp.tile([128, 128], f32, tag=f"osb{oh}")
                        nc.vector.tensor_scalar_add(
                            osb[:], cbM[:, 256 + oh * 128:384 + oh * 128],
                            b3_col_sb[oh][:])
                        nc.gpsimd.dma_start(
                            outT_d[oh * 128:(oh + 1) * 128,
                                   b * 128:b * 128 + nn],
                            osb[:, 0:nn])

    nc.compile()
    return nc


_CACHE = {}


def _get_nc():
    if "nc" not in _CACHE:
        _CACHE["nc"] = _build()
    return _CACHE["nc"]


def prepare(inputs):
    """Build (nc, in_maps) for the SPMD run."""
    src = np.asarray(inputs["src"], dtype=np.int32)
    dst = np.asarray(inputs["dst"], dtype=np.int32)
    rel = np.asarray(inputs["rel"], dtype=np.int32)
    com = _prep_weights(inputs)
    Cs = _prep_counts(src, dst, rel)
    nc = _get_nc()
    in_maps = []
    for c in range(NCORES):
        m = dict(com)
        m["C"] = Cs[c]
        in_maps.append(m)
    return nc, in_maps


# ----------------------------------------------------------------------------
# public entry
# ----------------------------------------------------------------------------

def kernel(**inputs) -> np.ndarray:
    nc, in_maps = prepare(inputs)
    res = bass_utils.run_bass_kernel_spmd(nc, in_maps, core_ids=list(range(NCORES)))
    out = np.empty((N_NODES, D_OUT), dtype=np.float32)
    for c in range(NCORES):
        out[c * NPC:(c + 1) * NPC, :] = res.results[c]["outT"].T
    return out

